# revision 8
# baseline (speedup 1.0000x reference)
"""Trainium2 Bass kernel for nn_EncodingNetwork (gnn_message_passing).

Math (exact collapse of the reference):
    enc       = x @ W_enc.T + b_enc                    [N=200, D=1024]
    cm[w]     = class-mean of enc = xm[w] @ W_enc.T + b_enc   (xm = class-mean of x)
    gm        = mean(enc, axis=0) = mean(cm, axis=0)
    per_class = cm @ Wl.T + gm @ Wr.T + b_rel          [20, 2D]
    out       = gaussian * per_class[:, D:] + per_class[:, :D]

Sharding across 8 cores: the final 1024 output columns are split 128/core.
Each core loads its 256-row slice of W_rel (1 MB bf16) plus either the full
W_enc (2 MB, no-collective variant) or only a 128-column slice (0.25 MB,
AllGather variant).  Everything on device runs in transposed
[feature, class] layout so the contraction dim always sits on SBUF
partitions.

v2 pipeline notes (from NTFF trace analysis of v1 @29.0us):
  - The two HWDGE rings (nc.sync=SP, nc.scalar=Act) share the 16 SDMA
    engines at packet granularity; aggregate ~424 GB/s.  SDMA engine 15
    (E79) straggles 25-50%, delaying any full-blob semaphore by 2-4 us.
    Fix: chunk wcm/wrel into per-consumption-step DMAs, alternate rings in
    consumption order, and run each cm/rel step as its chunk lands.
  - sml (biases+gaussian) loads FIRST: v1 loaded it last and the cm bias
    adds blocked on it for ~4 us.
  - The rhs/mean term is accumulated as 1-column matvecs against per-chunk
    class-sums (gsum, computed on the otherwise-idle DVE as each cm chunk
    retires) instead of 20-column products + end reductions.
  - xm PSUM pool gets 3 bufs (v1's 2 caused ~480ns/pair stalls).
  - The ~8us postamble (walrus per-engine sem-reset storm) is fixed cost.
"""

import numpy as np

import concourse.bass as bass  # noqa: F401
import concourse.tile as tile
from concourse import bacc, mybir
from concourse.bass import ts
from concourse.bass_utils import run_bass_kernel_spmd

N_WAY = 20
N_SUPPORT = 10
N = N_WAY * N_SUPPORT  # 200
D = 1024
NC = 8
SL = D // NC  # 128 output columns per core
KT = D // 128  # 8 contraction tiles
XW = D + N_WAY  # x | selector columns, per 128-row tile
F32 = mybir.dt.float32
BF16 = mybir.dt.bfloat16

USE_ALLGATHER = False
MM_DTYPE = "bf16"  # epilogue always fp32


def _build_nc(use_ag: bool) -> bacc.Bacc:
    nc = bacc.Bacc("TRN2", target_bir_lowering=False, debug=False, num_devices=NC)

    FD = {"f32": F32, "f32r": mybir.dt.float32r, "bf16": BF16}[MM_DTYPE]
    mm = nc.tensor.matmul

    wenc_w = SL if use_ag else D
    CCW = N_WAY + 1  # cm slice | gsum slice through the collective

    xs_h = nc.declare_dram_parameter("xsb", [128, 2 * XW], FD, isOutput=False)
    wcm_h = nc.declare_dram_parameter("wcm", [128, KT * wenc_w], FD, isOutput=False)
    wrel_h = nc.declare_dram_parameter("wrel", [128, KT * 512], FD, isOutput=False)
    sml_h = nc.declare_dram_parameter("sml", [128, 31], F32, isOutput=False)
    out_h = nc.declare_dram_parameter("out", [128, N_WAY], F32, isOutput=True)

    if use_ag:
        cc_in = nc.dram_tensor("cc_in", [128, CCW], FD)
        cc_out = nc.dram_tensor("cc_out", [D, CCW], FD, addr_space="Shared")

    with tile.TileContext(nc) as tc:
        with (
            tc.tile_pool(name="sbuf", bufs=1) as sb,
            tc.tile_pool(name="psA", bufs=2, space="PSUM") as psA,
            tc.tile_pool(name="psB", bufs=2, space="PSUM") as psB,
            tc.tile_pool(name="psR", bufs=1, space="PSUM") as psR,
        ):
            # ---- loads.  Ring FIFOs (HWDGE preserves per-engine order).
            # Only 8 DMAHW sem lanes exist; a 9th+ DMA reuses a lane and its
            # TRIGGER waits for the prior user's completion, so keep the DMA
            # count low and order lane reuses onto early-completing lanes.
            # Chunks land roughly in consumption order; each cm/rel step
            # waits only on its own chunk's semaphore.
            xs_all = sb.tile([128, 2 * XW], FD, tag="xs")
            nc.sync.dma_start(xs_all[:, :XW], xs_h[:, :XW])
            nc.scalar.dma_start(xs_all[:, XW:], xs_h[:, XW:])
            smw = sb.tile([128, 31], F32, tag="smw")
            nc.sync.dma_start(smw[:], sml_h[:])

            wcm_all = sb.tile([128, KT * wenc_w], FD, tag="wcm")
            wrel_all = sb.tile([128, KT * 512], FD, tag="wrel")
            if use_ag:
                nc.sync.dma_start(wcm_all[:], wcm_h[:])
                for k in range(0, KT, 2):
                    eng = nc.scalar if k % 4 == 0 else nc.sync
                    eng.dma_start(
                        wrel_all[:, k * 512 : (k + 2) * 512],
                        wrel_h[:, k * 512 : (k + 2) * 512],
                    )
            else:
                for t in range(0, KT, 2):
                    eng = nc.sync if t % 4 == 0 else nc.scalar
                    eng.dma_start(
                        wcm_all[:, t * D : (t + 2) * D], wcm_h[:, t * D : (t + 2) * D]
                    )
                for k in range(0, KT, 2):
                    eng = nc.sync if k % 4 == 0 else nc.scalar
                    eng.dma_start(
                        wrel_all[:, k * 512 : (k + 2) * 512],
                        wrel_h[:, k * 512 : (k + 2) * 512],
                    )

            # ---- stage 1: xm^T [d-chunk, w] = x^T @ S, per k-tile.
            xm_sb = sb.tile([128, KT * N_WAY], FD, tag="xm")
            for t in range(KT):
                p = psA.tile([128, N_WAY], F32, tag="xm_ps")
                for i in range(2):
                    mm(
                        p[:],
                        xs_all[:, i * XW + t * 128 : i * XW + (t + 1) * 128],
                        xs_all[:, i * XW + D : i * XW + D + N_WAY],
                        start=(i == 0),
                        stop=(i == 1),
                    )
                nc.vector.tensor_copy(xm_sb[:, ts(t, N_WAY)], p[:])

            # ---- stage 2: cm chunks (+b_enc, cast) and per-chunk class sums.
            if use_ag:
                # own slice only: cm[:, S_c]^T = W_enc[S_c,:] @ xm^T
                cio = sb.tile([128, CCW], FD, tag="cio")
                pcm = psB.tile([128, N_WAY], F32, tag="cm_ps")
                for kt in range(KT):
                    mm(
                        pcm[:],
                        wcm_all[:, ts(kt, SL)],
                        xm_sb[:, ts(kt, N_WAY)],
                        start=(kt == 0),
                        stop=(kt == KT - 1),
                    )
                # + b_enc[s] (per-partition), cast to bf16
                nc.vector.tensor_scalar(
                    cio[:, :N_WAY], pcm[:], 1.0, smw[:, 30:31],
                    op0=mybir.AluOpType.mult, op1=mybir.AluOpType.add,
                )
                with nc.allow_low_precision("20-wide bf16 class-sum; ~0.4% rel"):
                    nc.vector.reduce_sum(
                        cio[:, N_WAY : N_WAY + 1], cio[:, :N_WAY],
                        axis=mybir.AxisListType.X,
                    )
                nc.sync.dma_start(cc_in[:], cio[:])
                nc.gpsimd.collective_compute(
                    "AllGather",
                    mybir.AluOpType.bypass,
                    replica_groups=[list(range(NC))],
                    ins=[cc_in[:]],
                    outs=[cc_out[:]],
                )
                cmf_sb = sb.tile([128, KT * CCW], FD, tag="cmf")
                nc.sync.dma_start(
                    cmf_sb[:].rearrange("p (t w) -> p t w", t=KT),
                    cc_out[:].rearrange("(t p) w -> p t w", p=128),
                )

                def rel_rhs(t):
                    return cmf_sb[:, t * CCW : t * CCW + N_WAY]

                def rel_gs(t):
                    return cmf_sb[:, t * CCW + N_WAY : (t + 1) * CCW]
            else:
                cmf_sb = sb.tile([128, KT * N_WAY], FD, tag="cmf")
                gsum_sb = sb.tile([128, KT], FD, tag="gsum")
                for t in range(KT):
                    pcm = psB.tile([128, N_WAY], F32, tag="cm_ps")
                    for kt in range(KT):
                        mm(
                            pcm[:],
                            wcm_all[:, t * D + kt * 128 : t * D + (kt + 1) * 128],
                            xm_sb[:, ts(kt, N_WAY)],
                            start=(kt == 0),
                            stop=(kt == KT - 1),
                        )
                    # cmf = pcm + b_enc chunk (per-partition), cast to bf16
                    nc.vector.tensor_scalar(
                        cmf_sb[:, ts(t, N_WAY)], pcm[:], 1.0, smw[:, t : t + 1],
                        op0=mybir.AluOpType.mult, op1=mybir.AluOpType.add,
                    )
                    with nc.allow_low_precision("20-wide bf16 class-sum; ~0.4% rel"):
                        nc.vector.reduce_sum(
                            gsum_sb[:, t : t + 1], cmf_sb[:, ts(t, N_WAY)],
                            axis=mybir.AxisListType.X,
                        )

                def rel_rhs(t):
                    return cmf_sb[:, ts(t, N_WAY)]

                def rel_gs(t):
                    return gsum_sb[:, t : t + 1]

            # ---- stage 3: rel products.  pm/pstd: [128,20] over classes;
            # prs: [128,2] matvecs against the class-sums (mean term).
            pm = psR.tile([128, N_WAY], F32, tag="pm")
            pstd = psR.tile([128, N_WAY], F32, tag="pstd")
            prm = psR.tile([128, 1], F32, tag="prm")
            prstd = psR.tile([128, 1], F32, tag="prstd")
            for kt in range(KT):
                st, sp = (kt == 0), (kt == KT - 1)
                o = kt * 512
                rhs, gs = rel_rhs(kt), rel_gs(kt)
                mm(pm[:], wrel_all[:, o : o + 128], rhs, start=st, stop=sp)
                mm(prm[:], wrel_all[:, o + 128 : o + 256], gs, start=st, stop=sp)
                mm(pstd[:], wrel_all[:, o + 256 : o + 384], rhs, start=st, stop=sp)
                mm(prstd[:], wrel_all[:, o + 384 : o + 512], gs, start=st, stop=sp)

            # ---- stage 4: biases + gaussian combine
            bias_m = sb.tile([128, 1], F32, tag="bias_m")
            bias_s = sb.tile([128, 1], F32, tag="bias_s")
            nc.vector.tensor_scalar(
                bias_m[:], prm[:], 1.0 / N_WAY, smw[:, 8:9],
                op0=mybir.AluOpType.mult, op1=mybir.AluOpType.add,
            )
            nc.vector.tensor_scalar(
                bias_s[:], prstd[:], 1.0 / N_WAY, smw[:, 9:10],
                op0=mybir.AluOpType.mult, op1=mybir.AluOpType.add,
            )
            t_sg = sb.tile([128, N_WAY], F32, tag="t_sg")
            nc.vector.scalar_tensor_tensor(
                t_sg[:], pstd[:], bias_s[:], smw[:, 10:30],
                op0=mybir.AluOpType.add, op1=mybir.AluOpType.mult,
            )
            out_sb = sb.tile([128, N_WAY], F32, tag="out")
            nc.vector.scalar_tensor_tensor(
                out_sb[:], pm[:], bias_m[:], t_sg[:],
                op0=mybir.AluOpType.add, op1=mybir.AluOpType.add,
            )
            nc.sync.dma_start(out_h[:], out_sb[:])

    nc.finalize()
    return nc


_NC_CACHE: dict = {}


def _get_nc(use_ag: bool) -> bacc.Bacc:
    key = (use_ag, MM_DTYPE)
    if key not in _NC_CACHE:
        _NC_CACHE[key] = _build_nc(use_ag)
    return _NC_CACHE[key]


def _np_dtype():
    if MM_DTYPE == "bf16":
        import ml_dtypes

        return ml_dtypes.bfloat16
    return np.float32


def _make_in_maps(x, W_enc, b_enc, W_rel, b_rel, gaussian, use_ag):
    nd = _np_dtype()
    # The class-mean scaling 1/N_SUPPORT is folded into W_enc on the host
    # (in fp32, before any cast) so the selector stays exactly 1.0.
    W_enc = W_enc / np.float32(N_SUPPORT)
    # xsb: [128, 2*XW] — two 128-row tiles of [x | selector]
    xs = np.zeros((2, 128, XW), np.float32)
    xs[:, :, :D].reshape(256, D)[:N] = x
    sel = np.zeros((N, N_WAY), np.float32)
    sel[np.arange(N), np.arange(N) // N_SUPPORT] = 1.0
    xs[:, :, D : D + N_WAY].reshape(256, N_WAY)[:N] = sel

    in_maps = []
    for c in range(NC):
        s = slice(c * SL, (c + 1) * SL)
        s2 = slice(D + c * SL, D + (c + 1) * SL)
        if use_ag:
            # W_enc^T column slice: [D, SL] -> [128, KT*SL] chunk-interleaved
            wcm = (
                np.ascontiguousarray(W_enc[s, :].T)
                .reshape(KT, 128, SL)
                .transpose(1, 0, 2)
                .reshape(128, KT * SL)
            )
        else:
            # t-major blocks: wcm[p, t*D + kt*128 + j] = W_enc[t*128+j, kt*128+p]
            wcm = (
                np.ascontiguousarray(W_enc.T)
                .reshape(KT, 128, KT, 128)
                .transpose(1, 2, 0, 3)
                .reshape(128, KT * D)
            )
        blk = np.empty((KT, 128, 512), np.float32)
        for i, m in enumerate(
            (W_rel[s, :D], W_rel[s, D:], W_rel[s2, :D], W_rel[s2, D:])
        ):
            blk[:, :, i * 128 : (i + 1) * 128] = (
                np.ascontiguousarray(m.T).reshape(KT, 128, SL)
            )
        wrel = blk.transpose(1, 0, 2).reshape(128, KT * 512)

        sm = np.zeros((128, 31), np.float32)
        sm[:, 0:8] = b_enc.reshape(KT, 128).T
        sm[:, 8] = b_rel[s]
        sm[:, 9] = b_rel[s2]
        sm[:, 10:30] = gaussian[:, s].T
        sm[:, 30] = b_enc[s]
        in_maps.append(
            {
                "xsb": np.ascontiguousarray(
                    xs.transpose(1, 0, 2).reshape(128, -1)
                ).astype(nd),
                "wcm": np.ascontiguousarray(wcm).astype(nd),
                "wrel": np.ascontiguousarray(wrel).astype(nd),
                "sml": sm,
            }
        )
    return in_maps


def run(inputs: dict, trace: bool = False, use_ag: bool = USE_ALLGATHER):
    x = np.asarray(inputs["x_support"], np.float32)
    W_enc = np.asarray(inputs["W_enc"], np.float32)
    b_enc = np.asarray(inputs["b_enc"], np.float32)
    W_rel = np.asarray(inputs["W_rel"], np.float32)
    b_rel = np.asarray(inputs["b_rel"], np.float32)
    gaussian = np.asarray(inputs["gaussian_vectors"], np.float32)

    nc = _get_nc(use_ag)
    in_maps = _make_in_maps(x, W_enc, b_enc, W_rel, b_rel, gaussian, use_ag)
    res = run_bass_kernel_spmd(nc, in_maps, list(range(NC)), trace=trace)

    out = np.empty((N_WAY, D), np.float32)
    for c in range(NC):
        out[:, c * SL : (c + 1) * SL] = res.results[c]["out"].T
    return out, res


def kernel(**inputs) -> np.ndarray:
    out, _ = run(inputs)
    return out


# revision 11
# speedup vs baseline: 1.0291x; 1.0291x over previous
"""Trainium2 Bass kernel for nn_EncodingNetwork (gnn_message_passing).

Math (exact collapse of the reference):
    enc       = x @ W_enc.T + b_enc                    [N=200, D=1024]
    cm[w]     = class-mean of enc = xm[w] @ W_enc.T + b_enc   (xm = class-mean of x)
    gm        = mean(enc, axis=0) = mean(cm, axis=0)
    per_class = cm @ Wl.T + gm @ Wr.T + b_rel          [20, 2D]
    out       = gaussian * per_class[:, D:] + per_class[:, :D]

Sharding across 8 cores: the final 1024 output columns are split 128/core.
Each core loads its 256-row slice of W_rel (1 MB bf16) plus either the full
W_enc (2 MB, no-collective variant) or only a 128-column slice (0.25 MB,
AllGather variant).  Everything on device runs in transposed
[feature, class] layout so the contraction dim always sits on SBUF
partitions.

v2 pipeline notes (from NTFF trace analysis of v1 @29.0us):
  - The two HWDGE rings (nc.sync=SP, nc.scalar=Act) share the 16 SDMA
    engines at packet granularity; aggregate ~424 GB/s.  SDMA engine 15
    (E79) straggles 25-50%, delaying any full-blob semaphore by 2-4 us.
    Fix: chunk wcm/wrel into per-consumption-step DMAs, alternate rings in
    consumption order, and run each cm/rel step as its chunk lands.
  - sml (biases+gaussian) loads FIRST: v1 loaded it last and the cm bias
    adds blocked on it for ~4 us.
  - The rhs/mean term is accumulated as 1-column matvecs against per-chunk
    class-sums (gsum, computed on the otherwise-idle DVE as each cm chunk
    retires) instead of 20-column products + end reductions.
  - xm PSUM pool gets 3 bufs (v1's 2 caused ~480ns/pair stalls).
  - The ~8us postamble (walrus per-engine sem-reset storm) is fixed cost.
"""

import numpy as np

import concourse.bass as bass  # noqa: F401
import concourse.tile as tile
from concourse import bacc, mybir
from concourse.bass import ts
from concourse.bass_utils import run_bass_kernel_spmd

N_WAY = 20
N_SUPPORT = 10
N = N_WAY * N_SUPPORT  # 200
D = 1024
NC = 8
SL = D // NC  # 128 output columns per core
KT = D // 128  # 8 contraction tiles
XW = D + N_WAY  # x | selector columns, per 128-row tile
F32 = mybir.dt.float32
BF16 = mybir.dt.bfloat16

USE_ALLGATHER = False
MM_DTYPE = "bf16"  # epilogue always fp32


def _build_nc(use_ag: bool) -> bacc.Bacc:
    nc = bacc.Bacc("TRN2", target_bir_lowering=False, debug=False, num_devices=NC)

    FD = {"f32": F32, "f32r": mybir.dt.float32r, "bf16": BF16}[MM_DTYPE]
    mm = nc.tensor.matmul

    wenc_w = SL if use_ag else D
    CCW = N_WAY + 1  # cm slice | gsum slice through the collective

    # every chunk is its own contiguous DRAM tensor: strided slices of a
    # big tensor stall HWDGE descriptor generation for ~3us per chunk
    xsa_h = nc.declare_dram_parameter("xsa", [128, XW], FD, isOutput=False)
    xsb_h = nc.declare_dram_parameter("xsb", [128, XW], FD, isOutput=False)
    if use_ag:
        wcm_hs = [nc.declare_dram_parameter("wcm0", [128, KT * SL], FD, isOutput=False)]
    else:
        wcm_hs = [
            nc.declare_dram_parameter(f"wcm{i}", [128, 2 * D], FD, isOutput=False)
            for i in range(4)
        ]
    wrel_hs = [
        nc.declare_dram_parameter(f"wr{i}", [128, 1024], FD, isOutput=False)
        for i in range(4)
    ]
    sml_h = nc.declare_dram_parameter("sml", [128, 31], F32, isOutput=False)
    out_h = nc.declare_dram_parameter("out", [128, N_WAY], F32, isOutput=True)

    if use_ag:
        cc_in = nc.dram_tensor("cc_in", [128, CCW], FD)
        cc_out = nc.dram_tensor("cc_out", [D, CCW], FD, addr_space="Shared")

    with tile.TileContext(nc) as tc:
        with (
            tc.tile_pool(name="sbuf", bufs=1) as sb,
            tc.tile_pool(name="psA", bufs=2, space="PSUM") as psA,
            tc.tile_pool(name="psB", bufs=2, space="PSUM") as psB,
            tc.tile_pool(name="psR", bufs=1, space="PSUM") as psR,
        ):
            # ---- loads.  Ring FIFOs (HWDGE preserves per-engine order).
            # Only 8 DMAHW sem lanes exist; a 9th+ DMA reuses a lane and its
            # TRIGGER waits for the prior user's completion, so keep the DMA
            # count low and order lane reuses onto early-completing lanes.
            # Chunks land roughly in consumption order; each cm/rel step
            # waits only on its own chunk's semaphore.
            xs_all = sb.tile([128, 2 * XW], FD, tag="xs")
            nc.sync.dma_start(xs_all[:, :XW], xsa_h[:])
            nc.scalar.dma_start(xs_all[:, XW:], xsb_h[:])
            smw = sb.tile([128, 31], F32, tag="smw")
            nc.sync.dma_start(smw[:], sml_h[:])

            wcm_all = sb.tile([128, KT * wenc_w], FD, tag="wcm")
            wrel_all = sb.tile([128, KT * 512], FD, tag="wrel")
            if use_ag:
                nc.sync.dma_start(wcm_all[:], wcm_hs[0][:])
                for i in range(4):
                    eng = nc.scalar if i % 2 == 0 else nc.sync
                    eng.dma_start(
                        wrel_all[:, i * 1024 : (i + 1) * 1024], wrel_hs[i][:]
                    )
            else:
                for i in range(4):
                    eng = nc.sync if i % 2 == 0 else nc.scalar
                    eng.dma_start(
                        wcm_all[:, 2 * i * D : 2 * (i + 1) * D], wcm_hs[i][:]
                    )
                for i in range(4):
                    eng = nc.sync if i % 2 == 0 else nc.scalar
                    eng.dma_start(
                        wrel_all[:, i * 1024 : (i + 1) * 1024], wrel_hs[i][:]
                    )

            # ---- stage 1: xm^T [d-chunk, w] = x^T @ S, per k-tile.
            xm_sb = sb.tile([128, KT * N_WAY], FD, tag="xm")
            for t in range(KT):
                p = psA.tile([128, N_WAY], F32, tag="xm_ps")
                for i in range(2):
                    mm(
                        p[:],
                        xs_all[:, i * XW + t * 128 : i * XW + (t + 1) * 128],
                        xs_all[:, i * XW + D : i * XW + D + N_WAY],
                        start=(i == 0),
                        stop=(i == 1),
                    )
                nc.vector.tensor_copy(xm_sb[:, ts(t, N_WAY)], p[:])

            # ---- stage 2: cm chunks (+b_enc, cast) and per-chunk class sums.
            if use_ag:
                # own slice only: cm[:, S_c]^T = W_enc[S_c,:] @ xm^T
                cio = sb.tile([128, CCW], FD, tag="cio")
                pcm = psB.tile([128, N_WAY], F32, tag="cm_ps")
                for kt in range(KT):
                    mm(
                        pcm[:],
                        wcm_all[:, ts(kt, SL)],
                        xm_sb[:, ts(kt, N_WAY)],
                        start=(kt == 0),
                        stop=(kt == KT - 1),
                    )
                # + b_enc[s] (per-partition), cast to bf16
                nc.vector.tensor_scalar(
                    cio[:, :N_WAY], pcm[:], 1.0, smw[:, 30:31],
                    op0=mybir.AluOpType.mult, op1=mybir.AluOpType.add,
                )
                with nc.allow_low_precision("20-wide bf16 class-sum; ~0.4% rel"):
                    nc.vector.reduce_sum(
                        cio[:, N_WAY : N_WAY + 1], cio[:, :N_WAY],
                        axis=mybir.AxisListType.X,
                    )
                nc.sync.dma_start(cc_in[:], cio[:])
                nc.gpsimd.collective_compute(
                    "AllGather",
                    mybir.AluOpType.bypass,
                    replica_groups=[list(range(NC))],
                    ins=[cc_in[:]],
                    outs=[cc_out[:]],
                )
                cmf_sb = sb.tile([128, KT * CCW], FD, tag="cmf")
                nc.sync.dma_start(
                    cmf_sb[:].rearrange("p (t w) -> p t w", t=KT),
                    cc_out[:].rearrange("(t p) w -> p t w", p=128),
                )

                def rel_rhs(t):
                    return cmf_sb[:, t * CCW : t * CCW + N_WAY]

                def rel_gs(t):
                    return cmf_sb[:, t * CCW + N_WAY : (t + 1) * CCW]
            else:
                cmf_sb = sb.tile([128, KT * N_WAY], FD, tag="cmf")
                gsum_sb = sb.tile([128, KT], FD, tag="gsum")
                for t in range(KT):
                    pcm = psB.tile([128, N_WAY], F32, tag="cm_ps")
                    for kt in range(KT):
                        mm(
                            pcm[:],
                            wcm_all[:, t * D + kt * 128 : t * D + (kt + 1) * 128],
                            xm_sb[:, ts(kt, N_WAY)],
                            start=(kt == 0),
                            stop=(kt == KT - 1),
                        )
                    # cmf = pcm + b_enc chunk (per-partition), cast to bf16
                    nc.vector.tensor_scalar(
                        cmf_sb[:, ts(t, N_WAY)], pcm[:], 1.0, smw[:, t : t + 1],
                        op0=mybir.AluOpType.mult, op1=mybir.AluOpType.add,
                    )
                    with nc.allow_low_precision("20-wide bf16 class-sum; ~0.4% rel"):
                        nc.vector.reduce_sum(
                            gsum_sb[:, t : t + 1], cmf_sb[:, ts(t, N_WAY)],
                            axis=mybir.AxisListType.X,
                        )

                def rel_rhs(t):
                    return cmf_sb[:, ts(t, N_WAY)]

                def rel_gs(t):
                    return gsum_sb[:, t : t + 1]

            # ---- stage 3: rel products.  pm/pstd: [128,20] over classes;
            # prs: [128,2] matvecs against the class-sums (mean term).
            pm = psR.tile([128, N_WAY], F32, tag="pm")
            pstd = psR.tile([128, N_WAY], F32, tag="pstd")
            prm = psR.tile([128, 1], F32, tag="prm")
            prstd = psR.tile([128, 1], F32, tag="prstd")
            for kt in range(KT):
                st, sp = (kt == 0), (kt == KT - 1)
                o = kt * 512
                rhs, gs = rel_rhs(kt), rel_gs(kt)
                mm(pm[:], wrel_all[:, o : o + 128], rhs, start=st, stop=sp)
                mm(prm[:], wrel_all[:, o + 128 : o + 256], gs, start=st, stop=sp)
                mm(pstd[:], wrel_all[:, o + 256 : o + 384], rhs, start=st, stop=sp)
                mm(prstd[:], wrel_all[:, o + 384 : o + 512], gs, start=st, stop=sp)

            # ---- stage 4: biases + gaussian combine
            bias_m = sb.tile([128, 1], F32, tag="bias_m")
            bias_s = sb.tile([128, 1], F32, tag="bias_s")
            nc.vector.tensor_scalar(
                bias_m[:], prm[:], 1.0 / N_WAY, smw[:, 8:9],
                op0=mybir.AluOpType.mult, op1=mybir.AluOpType.add,
            )
            nc.vector.tensor_scalar(
                bias_s[:], prstd[:], 1.0 / N_WAY, smw[:, 9:10],
                op0=mybir.AluOpType.mult, op1=mybir.AluOpType.add,
            )
            t_sg = sb.tile([128, N_WAY], F32, tag="t_sg")
            nc.vector.scalar_tensor_tensor(
                t_sg[:], pstd[:], bias_s[:], smw[:, 10:30],
                op0=mybir.AluOpType.add, op1=mybir.AluOpType.mult,
            )
            out_sb = sb.tile([128, N_WAY], F32, tag="out")
            nc.vector.scalar_tensor_tensor(
                out_sb[:], pm[:], bias_m[:], t_sg[:],
                op0=mybir.AluOpType.add, op1=mybir.AluOpType.add,
            )
            nc.sync.dma_start(out_h[:], out_sb[:])

    nc.finalize()
    return nc


_NC_CACHE: dict = {}


def _get_nc(use_ag: bool) -> bacc.Bacc:
    key = (use_ag, MM_DTYPE)
    if key not in _NC_CACHE:
        _NC_CACHE[key] = _build_nc(use_ag)
    return _NC_CACHE[key]


def _np_dtype():
    if MM_DTYPE == "bf16":
        import ml_dtypes

        return ml_dtypes.bfloat16
    return np.float32


def _make_in_maps(x, W_enc, b_enc, W_rel, b_rel, gaussian, use_ag):
    nd = _np_dtype()
    # The class-mean scaling 1/N_SUPPORT is folded into W_enc on the host
    # (in fp32, before any cast) so the selector stays exactly 1.0.
    W_enc = W_enc / np.float32(N_SUPPORT)
    # xsb: [128, 2*XW] — two 128-row tiles of [x | selector]
    xs = np.zeros((2, 128, XW), np.float32)
    xs[:, :, :D].reshape(256, D)[:N] = x
    sel = np.zeros((N, N_WAY), np.float32)
    sel[np.arange(N), np.arange(N) // N_SUPPORT] = 1.0
    xs[:, :, D : D + N_WAY].reshape(256, N_WAY)[:N] = sel

    in_maps = []
    for c in range(NC):
        s = slice(c * SL, (c + 1) * SL)
        s2 = slice(D + c * SL, D + (c + 1) * SL)
        if use_ag:
            # W_enc^T column slice: [D, SL] -> [128, KT*SL] chunk-interleaved
            wcm = (
                np.ascontiguousarray(W_enc[s, :].T)
                .reshape(KT, 128, SL)
                .transpose(1, 0, 2)
                .reshape(128, KT * SL)
            )
        else:
            # t-major blocks: wcm[p, t*D + kt*128 + j] = W_enc[t*128+j, kt*128+p]
            wcm = (
                np.ascontiguousarray(W_enc.T)
                .reshape(KT, 128, KT, 128)
                .transpose(1, 2, 0, 3)
                .reshape(128, KT * D)
            )
        blk = np.empty((KT, 128, 512), np.float32)
        for i, m in enumerate(
            (W_rel[s, :D], W_rel[s, D:], W_rel[s2, :D], W_rel[s2, D:])
        ):
            blk[:, :, i * 128 : (i + 1) * 128] = (
                np.ascontiguousarray(m.T).reshape(KT, 128, SL)
            )
        wrel = blk.transpose(1, 0, 2).reshape(128, KT * 512)

        sm = np.zeros((128, 31), np.float32)
        sm[:, 0:8] = b_enc.reshape(KT, 128).T
        sm[:, 8] = b_rel[s]
        sm[:, 9] = b_rel[s2]
        sm[:, 10:30] = gaussian[:, s].T
        sm[:, 30] = b_enc[s]
        m = {
            "xsa": np.ascontiguousarray(xs[0]).astype(nd),
            "xsb": np.ascontiguousarray(xs[1]).astype(nd),
            "sml": sm,
        }
        if use_ag:
            m["wcm0"] = np.ascontiguousarray(wcm).astype(nd)
        else:
            for i in range(4):
                m[f"wcm{i}"] = np.ascontiguousarray(
                    wcm[:, 2 * i * D : 2 * (i + 1) * D]
                ).astype(nd)
        for i in range(4):
            m[f"wr{i}"] = np.ascontiguousarray(
                wrel[:, i * 1024 : (i + 1) * 1024]
            ).astype(nd)
        in_maps.append(m)
    return in_maps


def run(inputs: dict, trace: bool = False, use_ag: bool = USE_ALLGATHER):
    x = np.asarray(inputs["x_support"], np.float32)
    W_enc = np.asarray(inputs["W_enc"], np.float32)
    b_enc = np.asarray(inputs["b_enc"], np.float32)
    W_rel = np.asarray(inputs["W_rel"], np.float32)
    b_rel = np.asarray(inputs["b_rel"], np.float32)
    gaussian = np.asarray(inputs["gaussian_vectors"], np.float32)

    nc = _get_nc(use_ag)
    in_maps = _make_in_maps(x, W_enc, b_enc, W_rel, b_rel, gaussian, use_ag)
    res = run_bass_kernel_spmd(nc, in_maps, list(range(NC)), trace=trace)

    out = np.empty((N_WAY, D), np.float32)
    for c in range(NC):
        out[:, c * SL : (c + 1) * SL] = res.results[c]["out"].T
    return out, res


def kernel(**inputs) -> np.ndarray:
    out, _ = run(inputs)
    return out
